# revision 49
# baseline (speedup 1.0000x reference)
"""Multi-head attention Trainium2 kernel (Bass/Tile), 8-core SPMD.

Problem: B=2, S=2048, D=1024, H=16 heads of d=64.
Sharding: core c -> batch c//4, 4 heads starting at 4*(c%4).
Each core computes its heads' Q/K/V projections, attention, and the
partial output projection (transposed); host sums the 4 partials per
batch and adds bo.

Design notes:
  - All matmul operands are bf16 (halves input DMA; bf16 matmuls run
    1 cycle/row at any moving size); psum accumulation stays f32.
  - Scores are computed transposed (S^T[sk, sq]) per head pair and exp'd
    on the ACT engine straight into bf16 P^T tiles.
  - PV uses P^T chunks as the *stationary* operand and V' (V plus a ones
    column) as the 65-row moving operand, yielding U[sq, dh|rowsum] in
    psum.  This halves PE time for the PV stage vs streaming P^T.
  - Softmax rowsums land in psum column 64, so normalization is a plain
    per-partition tensor_scalar multiply (no PE broadcast matmuls); the
    normalized A is transposed back with PE transpose for the output
    projection; bf16 partials go to DRAM via staged SBUF copies (the
    final epilogue splits them across the then-idle ACT engine and DVE).
  - Emission is software-pipelined: scores run one stage ahead of PV so
    the ACT engine (exp is ~133us of work) stays fed; projection and
    epilogue work is split into small units drained under a per-stage
    cost budget between attention stages to fill PE slack.  Labeled
    barriers force any still-queued producer group to finish emitting
    before a consumer stage is emitted, at half-projection granularity
    (dht halves for Q/K, seq halves for V) to keep bursts small.
  - A dummy-matmul warm-up stream ramps the PE clock to max p-state
    while the first DMAs are still in flight.
"""

import numpy as np
import ml_dtypes

import concourse.bass as bass
import concourse.mybir as mybir
import concourse.tile as tile
from concourse import bacc
from concourse.bass_utils import run_bass_kernel_spmd

F32 = mybir.dt.float32
BF16 = mybir.dt.bfloat16
AF = mybir.ActivationFunctionType

B, S, D = 2, 2048, 1024
H, DH = 16, 64
NCORES = 8
HL = H // (NCORES // B)       # 4 heads per core
DL = HL * DH                  # 256 local projection dims
PAIRS = HL // 2               # 2 head pairs (packed into 128 partitions)
NKT = D // 128                # 8 contraction tiles for projections
SB = 512                      # seq block
NSB = S // SB                 # 4
SCALE = 0.125                 # 1/sqrt(64)

NPBF16 = ml_dtypes.bfloat16

LAST_EXEC_NS = None
_TRACE = False
_TRACE_KW = {}


def _bcast_part(ap, parts):
    """View `ap` with the partition dim replaced by a step-0 broadcast."""
    return bass.AP(tensor=ap.tensor, offset=ap.offset, ap=[[0, parts]] + list(ap.ap[1:]))


class _BG:
    """Queue of (pe_cost_ns, closure) units drained between attention stages.

    Labels mark group completion points; drain_until(label) force-emits
    everything up to the label so producer groups always precede their
    consumers in program order.
    """

    def __init__(self):
        from collections import deque
        self.q = deque()
        self.done = set()

    def push(self, units, label=None):
        self.q.extend(units)
        if label is not None:
            self.q.append((0, label))

    def _step(self):
        c, fn = self.q.popleft()
        if callable(fn):
            fn()
            return c
        self.done.add(fn)
        return 0

    def drain_budget(self, ns):
        while self.q and ns > 0:
            ns -= self._step()

    def drain_until(self, label):
        if label in self.done:
            return
        while self.q:
            c, fn = self.q[0]
            self._step()
            if not callable(fn) and fn == label:
                return

    def drain_all(self):
        while self.q:
            self._step()


def _emit(tc, nc, t):
    import contextlib

    ctx = contextlib.ExitStack()
    with ctx:
        const = ctx.enter_context(tc.tile_pool(name="const", bufs=1))
        persist = ctx.enter_context(tc.tile_pool(name="persist", bufs=1))
        xin = ctx.enter_context(tc.tile_pool(name="xin", bufs=2))
        ptp = ctx.enter_context(tc.tile_pool(name="ptp", bufs=8))
        misc = ctx.enter_context(tc.tile_pool(name="misc", bufs=3))
        outp = ctx.enter_context(tc.tile_pool(name="outp", bufs=4))
        # PSUM budget (8 banks): stt 2x[128,1024]=4, kv 2x[128,512]=2,
        # ut 2x[128,4,128]=2.
        p_big = ctx.enter_context(tc.tile_pool(name="p_big", bufs=2, space="PSUM"))
        p_kv = ctx.enter_context(tc.tile_pool(name="p_kv", bufs=2, space="PSUM"))
        p_u = ctx.enter_context(tc.tile_pool(name="p_u", bufs=2, space="PSUM"))

        # ---------- PE clock warm-up (memsets first on the DVE queue) ----------
        wrm_w = persist.tile([128, 128], BF16)
        nc.vector.memset(wrm_w, 0.0)
        wrm_x = persist.tile([128, SB], BF16)
        nc.vector.memset(wrm_x, 0.0)
        wrm_ps = p_kv.tile([128, SB], F32, name="wrm_ps", tag="kv")
        for _ in range(11):
            nc.tensor.matmul(wrm_ps, wrm_w, wrm_x, start=True, stop=True)

        # ---------- persistent activations ----------
        qT = persist.tile([128, PAIRS, S], BF16)   # [dh-in-pair, pair, sq]
        kT = persist.tile([128, PAIRS, S], BF16)
        v_sb = persist.tile([128, 4 * NSB, HL, DH + 1], BF16)  # [sk, skt, head, d|1]
        nc.vector.memset(v_sb, 1.0)
        uacc = persist.tile([128, HL, 4 * NSB, DH + 1], F32)  # [sq, head, sqc, d|rowsum]
        aT = persist.tile([128, 2, S], BF16)       # [hd-in-jt, jt, sq]
        warm_src = persist.tile([1, 1], F32)
        nc.vector.memset(warm_src, 1.0)
        warm = persist.tile([1, 1], F32)
        nc.scalar.activation(warm, warm_src, AF.Exp)

        # ---------- weight/const tiles (DMAs emitted in the prologue) ----------
        # wq/wk live dht-major ([128, dht, kt, 128]) so each half is one
        # contiguous-2KB-row DMA; the host pre-packs them that way.
        wk_sb = const.tile([128, 2, NKT, 128], BF16)
        wq_sb = const.tile([128, 2, NKT, 128], BF16)
        wv_sb = const.tile([128, NKT, DL], BF16)
        wo_sb = const.tile([128, 2, D], BF16)
        bk_sb = const.tile([128, 2], F32)
        bq_sb = const.tile([128, 2], F32)
        bvb = const.tile([128, DL], F32)
        ident = const.tile([128, 128], BF16)

        # ---------- proj group builders (unit lists for bg queue) ----------
        MM1, MM2, EVAC, DMAU, TPU, OPMM = 215, 430, 60, 80, 120, 215

        def x_dma_units(xt, name, s0):
            """Load x[*, s0:s0+SB] as [128, NKT, SB] in two kt-halves."""
            st = {}

            def dma(half):
                def u():
                    if half == 0:
                        # All x DMAs are issued from the prologue, so all four
                        # blocks of each tensor are resident at once.
                        st["x"] = xin.tile([128, NKT, SB], BF16, name=name, tag=name[:2],
                                           bufs=4)
                    h0 = half * (NKT // 2)
                    nc.sync.dma_start(
                        out=st["x"][:, h0:h0 + NKT // 2, :],
                        in_=t[xt][h0 * 128:(h0 + NKT // 2) * 128, s0:s0 + SB].rearrange(
                            "(t p) s -> p t s", p=128))
                return u

            return st, [(DMAU, dma(0)), (DMAU, dma(1))]

        def qk_half_groups(which, idx):
            """Q/K projection for block idx, split into dht halves.
            Returns (units_dht0, units_dht1); scores for pair pr2 only need
            the dht==pr2 half."""
            s0 = idx * SB
            xt = {"q": "xqT", "k": "xkT"}[which]
            w_sb = {"q": wq_sb, "k": wk_sb}[which]
            b_sb = {"q": bq_sb, "k": bk_sb}[which]
            dst = {"q": qT, "k": kT}[which]
            st, dmas = x_dma_units(xt, f"x{which}_{idx}", s0)

            def half(dht, with_dma):
                units = list(dmas) if with_dma else []

                def mk(kt):
                    def u():
                        if kt == 0:
                            st[f"ps{dht}"] = p_kv.tile([128, SB], F32,
                                                       name=f"{which}ps_{idx}_{dht}", tag="kv")
                        nc.tensor.matmul(st[f"ps{dht}"], w_sb[:, dht, kt, :],
                                         st["x"][:, kt, :], start=(kt == 0), stop=(kt == NKT - 1))
                    return u

                units += [(MM1, mk(kt)) for kt in range(NKT)]

                def ev():
                    nc.vector.tensor_scalar_add(dst[:, dht, s0:s0 + SB], st[f"ps{dht}"],
                                                b_sb[:, dht:dht + 1])
                units.append((EVAC, ev))
                return units

            return half(0, True), half(1, False)

        def v_half_groups(sb):
            """V projection for block sb, split into seq halves.
            Half g covers seq sub-chunks ss=2g,2g+1 (key tiles skt=4sb+2g..)."""
            s0 = sb * SB
            st, dmas = x_dma_units("xvT", f"xv_{sb}", s0)

            def half(grp, with_dma):
                units = list(dmas) if with_dma else []

                def mk(kt):
                    def u():
                        if kt == 0:
                            st[f"ps{grp}"] = p_kv.tile([128, SB], F32,
                                                       name=f"vps_{sb}_{grp}", tag="kv")
                        for half_ in range(2):
                            ss = grp * 2 + half_
                            nc.tensor.matmul(st[f"ps{grp}"][:, half_ * DL:(half_ + 1) * DL],
                                             st["x"][:, kt, ss * 128:(ss + 1) * 128],
                                             wv_sb[:, kt, :],
                                             start=(kt == 0 and half_ == 0),
                                             stop=(kt == NKT - 1),
                                             skip_group_check=True)
                    return u

                units += [(MM2, mk(kt)) for kt in range(NKT)]

                def ev(half_):
                    def u():
                        skt = sb * 4 + grp * 2 + half_
                        nc.vector.tensor_add(
                            v_sb[:, skt, :, 0:DH],
                            st[f"ps{grp}"][:, half_ * DL:(half_ + 1) * DL].rearrange(
                                "p (h d) -> p h d", h=HL),
                            bvb.rearrange("p (h d) -> p h d", h=HL))
                    return u

                units += [(EVAC, ev(0)), (EVAC, ev(1))]
                return units

            return half(0, True), half(1, False)

        def epi_half_group(isq, pr2, last=False):
            """Normalize+transpose heads of pair `pr2` for query block isq;
            the pr2==1 half also runs the output projection (reads both jt)
            and DMAs partials to DRAM straight out of psum."""
            q0 = isq * SB
            st = {"A": {}}
            units = []

            def norm(sqc):
                def u():
                    sqg = isq * 4 + sqc
                    A = misc.tile([128, 2, DH], BF16, name="Anrm", tag="A")
                    rinv = misc.tile([128, 2], F32, name="rinv", tag="rinv")
                    nc.vector.reciprocal(rinv, uacc[:, 2 * pr2:2 * pr2 + 2, sqg, DH])
                    for hh in range(2):
                        nc.vector.tensor_scalar_mul(A[:, hh, :],
                                                    uacc[:, 2 * pr2 + hh, sqg, 0:DH],
                                                    rinv[:, hh:hh + 1])
                    st["A"][sqc] = A
                return u

            def tp(sqc):
                def u():
                    sqg = isq * 4 + sqc
                    tpp = p_kv.tile([128, 128], BF16, name="tpp", tag="kv")
                    nc.tensor.transpose(tpp, st["A"][sqc], ident)
                    nc.vector.tensor_copy(aT[:, pr2, sqg * 128:(sqg + 1) * 128], tpp)
                return u

            for sqc in range(4):
                units += [(EVAC, norm(sqc))]
            for sqc in range(4):
                units += [(TPU, tp(sqc))]
            if pr2 == 1:
                def opmm(mg, mi, jt):
                    def u():
                        if mi == 0 and jt == 0:
                            # After the final PV the p_u banks are dead, so the
                            # last epilogue rotates op tiles over both psum
                            # pools (4 slots) for a deeper outproj pipeline.
                            pool = p_u if (last and mg % 2) else p_kv
                            tag = "ut" if (last and mg % 2) else "kv"
                            st[f"op{mg}"] = [pool.tile([128, SB], F32,
                                                       name=f"op_{isq}_{mg}_{i}", tag=tag)
                                             for i in range(2)]
                        mt = mg * 2 + mi
                        nc.tensor.matmul(st[f"op{mg}"][mi],
                                         wo_sb[:, jt, mt * 128:(mt + 1) * 128],
                                         aT[:, jt, q0:q0 + SB], start=(jt == 0), stop=(jt == 1))
                    return u

                def opev(mg):
                    def u():
                        for mi in range(2):
                            mt = mg * 2 + mi
                            ot = outp.tile([128, SB], BF16, name="ot", tag="ot")
                            if last and mi == 0:
                                nc.scalar.copy(ot, st[f"op{mg}"][mi])
                            else:
                                nc.vector.tensor_copy(ot, st[f"op{mg}"][mi])
                            nc.sync.dma_start(out=t[f"outT{mt}"][:, q0:q0 + SB], in_=ot)
                    return u

                for mg in range(4):
                    units += [(OPMM, opmm(mg, 0, 0)), (OPMM, opmm(mg, 1, 0)),
                              (OPMM, opmm(mg, 0, 1)), (OPMM, opmm(mg, 1, 1)),
                              (EVAC, opev(mg))]
            return units

        # ---------- attention stage machinery ----------
        pts = {}
        ust = {}

        def emit_scores_exp(sb, stg):
            isq, pr2, skt = stg
            q0 = isq * SB
            sg = sb * 4 + skt
            stt = p_big.tile([128, 2 * SB], F32, name="stt", tag="big")
            for hi in range(2):
                od = hi * DH
                nc.tensor.matmul(stt[:, hi * SB:(hi + 1) * SB],
                                 kT[od:od + DH, pr2, sg * 128:(sg + 1) * 128],
                                 qT[od:od + DH, pr2, q0:q0 + SB],
                                 start=True, stop=True)
            pt = ptp.tile([128, 2 * SB], BF16, name="pt", tag="pt")
            nc.scalar.activation(pt, stt, AF.Exp, scale=SCALE)
            pts[stg] = pt

        def emit_pv(sb, stg, bg):
            isq, pr2, skt = stg
            pt = pts.pop(stg)
            sg = sb * 4 + skt
            if skt == 0:
                ust[(isq, pr2)] = [p_u.tile([128, 4, 128], F32,
                                            name=f"u_{sb}_{isq}_{pr2}_{hi}", tag="ut")
                                   for hi in range(2)]
            u2 = ust[(isq, pr2)]
            for hi in range(2):
                h = pr2 * 2 + hi
                for sqc in range(4):
                    nc.tensor.matmul(u2[hi][:, sqc, 0:DH + 1],
                                     pt[:, hi * SB + sqc * 128:hi * SB + (sqc + 1) * 128],
                                     v_sb[:, sg, h, :],
                                     start=(skt == 0 and sqc == 0), stop=(skt == 3),
                                     skip_group_check=True)
            if skt == 3:
                for hi in range(2):
                    h = pr2 * 2 + hi
                    sl = uacc[:, h, isq * 4:(isq + 1) * 4, :]
                    if sb == 0:
                        nc.vector.tensor_copy(sl, u2[hi][:, :, 0:DH + 1])
                    else:
                        nc.vector.tensor_add(sl, sl, u2[hi][:, :, 0:DH + 1])
                del ust[(isq, pr2)]
                if sb == NSB - 1:
                    bg.push(epi_half_group(isq, pr2, last=(isq == NSB - 1 and pr2 == 1)))

        def attention_sb(sb, bg, pr2_outer=False):
            # pr2-outer order defers the dht1 (pair-1) projection needs by 16
            # stages — used for sb0 where everything is still being loaded.
            if pr2_outer:
                stages = [(isq, pr2, skt)
                          for pr2 in range(PAIRS) for isq in range(NSB) for skt in range(4)]
            else:
                stages = [(isq, pr2, skt)
                          for isq in range(NSB) for pr2 in range(PAIRS) for skt in range(4)]
            budget = 700 if sb == NSB - 1 else 400
            # PV lags `depth` stages behind scores (bounded by the pt pool);
            # a deep lag at sb0 lets the ACT engine run on buffered scores
            # while the first V projection is still being forced in.
            depth = 2 if sb == NSB - 1 else 5
            from collections import deque
            pend = deque()

            def pop_pv():
                stg_ = pend.popleft()
                if stg_[2] == 0:
                    bg.drain_until(f"v{sb}a")
                elif stg_[2] == 2:
                    bg.drain_until(f"v{sb}b")
                emit_pv(sb, stg_, bg)

            for stg in stages:
                isq, pr2, skt = stg
                if skt == 0:
                    bg.drain_until(f"q{isq}{'ab'[pr2]}")
                    bg.drain_until(f"k{sb}{'ab'[pr2]}")
                emit_scores_exp(sb, stg)
                pend.append(stg)
                if len(pend) > depth:
                    pop_pv()
                bg.drain_budget(budget)
            while pend:
                pop_pv()
                bg.drain_budget(budget)

        # ---------- prologue ----------
        # DMA order drives the serialized DMA engine: K path first, then Q
        # (gates the first exp), then the remaining weight halves and V.
        nc.sync.dma_start(out=wk_sb[:, 0], in_=t["wkT"][:, 0])
        nc.sync.dma_start(out=bk_sb, in_=t["bk"].rearrange("(t p) -> p t", p=128))
        k0a, k0b = qk_half_groups("k", 0)
        q0a, q0b = qk_half_groups("q", 0)
        v0a, v0b = v_half_groups(0)
        for _, u in k0a[:2]:
            u()           # xk DMAs
        nc.sync.dma_start(out=wq_sb[:, 0], in_=t["wqT"][:, 0])
        nc.sync.dma_start(out=bq_sb, in_=t["bq"].rearrange("(t p) -> p t", p=128))
        for _, u in q0a[:2]:
            u()           # xq DMAs
        nc.sync.dma_start(out=wk_sb[:, 1], in_=t["wkT"][:, 1])
        nc.sync.dma_start(out=wq_sb[:, 1], in_=t["wqT"][:, 1])
        nc.sync.dma_start(out=wv_sb, in_=t["wvT"].rearrange("(t p) d -> p t d", p=128))
        nc.sync.dma_start(out=bvb, in_=_bcast_part(t["bv"].rearrange("(o d) -> o d", o=1), 128))
        for _, u in v0a[:2]:
            u()           # xv DMAs
        nc.sync.dma_start(out=wo_sb, in_=t["woT"].rearrange("(t p) m -> p t m", p=128))
        nc.sync.dma_start(out=ident, in_=t["ident"])
        for _, u in k0a[2:]:
            u()
        for _, u in q0a[2:]:
            u()

        # Background queue in order of first need.
        bg = _BG()
        bg.done.update({"k0a", "q0a"})
        qg = {i: qk_half_groups("q", i) for i in range(1, NSB)}
        kg = {i: qk_half_groups("k", i) for i in range(1, NSB)}
        vg = {i: v_half_groups(i) for i in range(1, NSB)}
        # Issue the remaining x DMAs now, in need order, so the (serialized)
        # DMA engine works ahead of the compute stream.
        for i in range(1, NSB):
            for _, u in qg[i][0][:2]:
                u()
        for i in range(1, NSB):
            for _, u in kg[i][0][:2]:
                u()
            for _, u in vg[i][0][:2]:
                u()
        # k0b/q0b first: their weight DMAs land early while the V path is
        # still waiting on xv, and they unblock the pair-1 stages (stage 5+).
        bg.push(k0b, "k0b")
        bg.push(q0b, "q0b")
        bg.push(v0a[2:], "v0a")
        bg.push(v0b, "v0b")
        bg.push(qg[1][0][2:], "q1a")
        bg.push(qg[1][1], "q1b")
        bg.push(qg[2][0][2:], "q2a")
        bg.push(qg[2][1], "q2b")
        bg.push(qg[3][0][2:], "q3a")
        bg.push(qg[3][1], "q3b")
        bg.push(kg[1][0][2:], "k1a")
        bg.push(vg[1][0][2:], "v1a")
        bg.push(kg[1][1], "k1b")
        bg.push(vg[1][1], "v1b")
        attention_sb(0, bg)
        bg.push(kg[2][0][2:], "k2a")
        bg.push(vg[2][0][2:], "v2a")
        bg.push(kg[2][1], "k2b")
        bg.push(vg[2][1], "v2b")
        attention_sb(1, bg)
        bg.push(kg[3][0][2:], "k3a")
        bg.push(vg[3][0][2:], "v3a")
        bg.push(kg[3][1], "k3b")
        bg.push(vg[3][1], "v3b")
        attention_sb(2, bg)
        attention_sb(3, bg)
        bg.drain_all()


def build():
    nc = bacc.Bacc("TRN2", target_bir_lowering=False, debug=False, num_devices=NCORES)
    t = {}
    for name, shape in [("xqT", [D, S]), ("xkT", [D, S]), ("xvT", [D, S]),
                        ("wqT", [128, 2, NKT, 128]), ("wkT", [128, 2, NKT, 128]),
                        ("wvT", [D, DL]),
                        ("woT", [DL, D]), ("ident", [128, 128])]:
        t[name] = nc.dram_tensor(name, shape, BF16, kind="ExternalInput").ap()
    for name, shape in [("bq", [DL]), ("bk", [DL]), ("bv", [DL])]:
        t[name] = nc.dram_tensor(name, shape, F32, kind="ExternalInput").ap()
    for mt in range(8):
        t[f"outT{mt}"] = nc.dram_tensor(f"outT{mt}", [128, S], BF16,
                                        kind="ExternalOutput").ap()
    with tile.TileContext(nc) as tc:
        _emit(tc, nc, t)
    nc.compile()
    return nc


def shard(inputs):
    q = np.ascontiguousarray(np.asarray(inputs["query"], dtype=np.float32))
    k = np.ascontiguousarray(np.asarray(inputs["key"], dtype=np.float32))
    v = np.ascontiguousarray(np.asarray(inputs["value"], dtype=np.float32))
    Wq = np.asarray(inputs["Wq"], dtype=np.float32)
    Wk = np.asarray(inputs["Wk"], dtype=np.float32)
    Wv = np.asarray(inputs["Wv"], dtype=np.float32)
    Wo = np.asarray(inputs["Wo"], dtype=np.float32)
    bq = np.asarray(inputs["bq"], dtype=np.float32)
    bk = np.asarray(inputs["bk"], dtype=np.float32)
    bv = np.asarray(inputs["bv"], dtype=np.float32)

    def bf(a):
        return np.ascontiguousarray(a).astype(NPBF16)

    def pack_dht(Wl):
        # [p, dht, kt, j] = Wl[dht*128+j, kt*128+p]
        return bf(Wl.reshape(2, 128, NKT, 128).transpose(3, 0, 2, 1))

    ident = np.eye(128, dtype=NPBF16)
    xT = [(bf(q[b].T), bf(k[b].T), bf(v[b].T)) for b in range(B)]
    maps = []
    for c in range(NCORES):
        b, hb = divmod(c, NCORES // B)
        js = slice(hb * DL, (hb + 1) * DL)
        xq, xk, xv = xT[b]
        maps.append({
            "xqT": xq, "xkT": xk, "xvT": xv,
            "wqT": pack_dht(Wq[js]),
            "wkT": pack_dht(Wk[js]),
            "wvT": bf(Wv[js].T),
            "woT": bf(Wo[:, js].T),
            "bq": np.ascontiguousarray(bq[js]),
            "bk": np.ascontiguousarray(bk[js]),
            "bv": np.ascontiguousarray(bv[js]),
            "ident": ident,
        })
    return maps


def unshard(results, inputs):
    bo = np.asarray(inputs["bo"], dtype=np.float32)
    out = np.empty((B, S, D), np.float32)
    g = NCORES // B
    for b in range(B):
        def full(r):
            return np.concatenate(
                [np.asarray(r[f"outT{mt}"]).astype(np.float32) for mt in range(8)], axis=0)
        acc = full(results[b * g])
        for i in range(1, g):
            acc += full(results[b * g + i])
        out[b] = acc.T + bo
    return out


def kernel(**inputs):
    global LAST_EXEC_NS
    nc = build()
    maps = shard(inputs)
    res = run_bass_kernel_spmd(nc, maps, core_ids=list(range(NCORES)),
                               trace=_TRACE, **_TRACE_KW)
    LAST_EXEC_NS = res.exec_time_ns
    return unshard(res.results, inputs)
